# revision 14
# baseline (speedup 1.0000x reference)
"""CubicFeatureSampling Trainium2 kernel.

Full-input contract: kernel(ptcloud, cubic_features, neighborhood_size) with
  ptcloud:        [B=4, N=8192, 3]   f32 in [-1, 1]
  cubic_features: [B=4, C=256, S=32, S, S] f32
  neighborhood_size: 1
returns [B, N, K=8, C] f32 (rel err ~8e-3 vs the f32 jax reference,
well inside the 2e-2 gate).

Strategy (8 NeuronCores): data-parallel over (batch, half-of-N); each core
handles 4096 points against its batch's feature volume. The kernel is pure
HBM byte movement, so both optimizations shrink bytes:

1. int8 corner-blocked table. The host re-lays the volume as a zero-padded,
   channel-last table  table[(x*S+y)*S+z] = [f(x+dx,y+dy,z+dz) for
   k=dx*4+dy*2+dz], quantized to int8 with one f32 scale per row
   (scale = max|row|/127), shape [32768, 2048] (2KB rows; 0 where a coord
   hits S). Each point's whole [8, C] output block is ONE 2KB row, already
   in the reference's corner order; out-of-bounds corners are exact zeros.
   The host dequantizes (out_i8 * scale[row]) while unsharding.

2. Sorted-block replication. The point clouds are heavily clustered
   (~144 unique cells per 4096-point shard, ~28 points/cell), so the host
   sorts points by cell and packs them into G blocks of P=6 points that
   share one table row (G <= 768 = T*128 with T=6; padded blocks repeat
   the row and are simply ignored when unsharding). The device then runs,
   per tile t of 128 blocks:
     - ONE indirect SWDGE gather (the HW-verified "offset [128,1] + flat
       dest [128, X]" form): partition p <- table row asg[p, t] (2KB),
     - ONE HWDGE store replicating each partition's 2KB row P times into
       the block-sorted output srt[(t*128+p)*P + r] via a 0-stride
       broadcast source AP.
   That is 6 gathers (768 descriptors) + 6 stores per core: ~1.6 MB read +
   ~9.4 MB written vs 8.4+8.4 MB for a per-point gather, and ~10x fewer
   SWDGE descriptors (whose ~10ns/descriptor emission was the previous
   bottleneck). The host scatters srt rows back to per-point order while
   dequantizing (pure unshard-side bookkeeping).

If a shard is NOT clustered enough (G > 768), fall back to the per-point
indirect-gather kernel (32 gathers of [128,1] + 4 stores, 78 us).
"""

import numpy as np

import concourse.bass as bass
import concourse.tile as tile
from concourse import mybir
from concourse.bass_utils import run_bass_kernel_spmd

# Problem constants (hardcoded per harness contract).
B = 4
N = 8192
C = 256
S = 32
K = 8
N_CORES = 8
NP = (B * N) // N_CORES   # points per core = 4096

TR = S * S * S            # table rows (32768)
ROW = K * C               # 2048 elements (2KB in int8) per table row

# Block-replication path constants.
P = 6                     # points per block (replication factor)
T = 6                     # tiles of 128 blocks
G_CAP = T * 128           # max blocks per core (768)

# Per-point fallback path constants.
PTS_PER_PART = NP // 128  # 32 points per partition
UPG = 8                   # points (gather units) per store group
GROUPS = PTS_PER_PART // UPG  # 4 groups

F32 = mybir.dt.float32
I8 = mybir.dt.int8
I32 = mybir.dt.int32


def _legalize_single_wait(nc):
    """The walrus build here accepts exactly ONE sync wait per instruction
    (setupSyncWait: 'Too many sync wait commands'), but Tile's add_semaphores
    pass emits up to ~3 on DMAs and the tail drain. Hoist all but the last
    wait of each instruction into standalone same-engine InstEventSemaphore
    waits placed immediately before it — semantically identical (the engine
    queue is processed in order)."""
    f = nc.m.functions[0]
    for b in f.blocks:
        out, changed = [], False
        for inst in b.instructions:
            si = inst.sync_info
            if si is not None and si.on_wait and len(si.on_wait) > 1:
                waits = list(si.on_wait)
                for w in waits[:-1]:
                    ev = mybir.InstEventSemaphore(
                        name=nc.get_next_instruction_name(), ins=[], outs=[])
                    ev.engine = inst.engine
                    ev.sync_info = mybir.SyncInfo(on_wait=[w], on_update=[])
                    nc.register_instruction(ev, overwrite=True)
                    out.append(ev)
                inst.sync_info = mybir.SyncInfo(
                    on_wait=[waits[-1]], on_update=list(si.on_update or []))
                changed = True
            out.append(inst)
        if changed:
            b.instructions = out


def build_bass_blocks():
    """Sorted-block replication kernel: T gathers + T replicating stores."""
    nc = bass.Bass("TRN2")
    table = nc.declare_dram_parameter("table", [TR, ROW], I8, isOutput=False)
    asg = nc.declare_dram_parameter("asg", [128, T], I32, isOutput=False)
    srt = nc.declare_dram_parameter("srt", [G_CAP * P, ROW], I8,
                                    isOutput=True)

    # Block g = t*128 + p occupies srt rows [g*P, (g+1)*P): tile t's
    # 128*P rows are contiguous, P*ROW bytes per partition.

    H = P // 2                # store-half replication count (3)

    with tile.TileContext(nc) as tc:
        with (
            tc.tile_pool(name="gather", bufs=T) as gpool,
            tc.tile_pool(name="rep", bufs=3) as rpool,
            tc.tile_pool(name="idx", bufs=1) as ipool,
        ):
            asg_sb = ipool.tile([128, T], I32, tag="asg")
            nc.sync.dma_start(out=asg_sb[:], in_=asg[:])

            # All T gathers up front (only 2KB/partition each): SWDGE
            # descriptor emission finishes before DVE replication starts,
            # avoiding GpSimd<->DVE SBUF descriptor-ring contention.
            gts = []
            for t in range(T):
                gt = gpool.tile([128, ROW], I8, tag="gt")
                nc.gpsimd.indirect_dma_start(
                    out=gt[:],
                    out_offset=None,
                    in_=table[:],
                    in_offset=bass.IndirectOffsetOnAxis(
                        ap=asg_sb[:, t:t + 1], axis=0),
                )
                gts.append(gt)

            for t in range(T):
                # Replicate each partition's 2KB row P times in SBUF (DVE,
                # i32-bitcast for 4x fewer elements) so the stores move
                # contiguous 3-row (6KB) descriptors at full engine rate.
                gt6 = rpool.tile([128, P * ROW], I8, tag="gt6")
                nc.vector.tensor_copy(
                    gt6[:].bitcast(I32).rearrange(
                        "p (r d) -> p r d", r=P),
                    gts[t][:].bitcast(I32).unsqueeze(1).broadcast_to(
                        (128, P, ROW // 4)))
                # Tile t's srt rows, viewed [p][r][d]; store the two
                # 3-row halves on the two HWDGE queues for finer overlap.
                srt_t = srt[t * 128 * P:(t + 1) * 128 * P, :].rearrange(
                    "(p r) d -> p r d", p=128)
                nc.sync.dma_start(
                    out=srt_t[:, 0:H, :], in_=gt6[:, 0:H * ROW])
                nc.scalar.dma_start(
                    out=srt_t[:, H:P, :], in_=gt6[:, H * ROW:P * ROW])

    _legalize_single_wait(nc)
    return nc


def build_bass_points():
    """Per-point fallback: 32 indirect gathers + 4 stores, on-device index
    computation from pts."""
    nc = bass.Bass("TRN2")
    pts = nc.declare_dram_parameter("pts", [NP, 3], F32, isOutput=False)
    table = nc.declare_dram_parameter("table", [TR, ROW], I8, isOutput=False)
    out = nc.declare_dram_parameter("out", [NP * K, C], I8, isOutput=True)

    outv = out[:].rearrange("(p u) d -> p (u d)", p=128)  # [128, 256*C]

    with tile.TileContext(nc) as tc:
        with (
            tc.tile_pool(name="gather", bufs=2) as gpool,
            tc.tile_pool(name="idx", bufs=1) as ipool,
        ):
            pt_all = ipool.tile([128, PTS_PER_PART * 3], F32, tag="ptall")
            nc.sync.dma_start(
                out=pt_all[:],
                in_=pts[:].rearrange("(p w) t -> p (w t)", p=128))

            # t = pt*16 + 16 (one f32 rounding, identical to the reference)
            t = ipool.tile([128, PTS_PER_PART * 3], F32, tag="t")
            nc.scalar.activation(
                out=t[:], in_=pt_all[:],
                func=mybir.ActivationFunctionType.Copy,
                bias=float(S) / 2.0, scale=float(S) / 2.0)

            # exact floor(t), robust to the f32->i32 rounding mode
            gi = ipool.tile([128, PTS_PER_PART * 3], I32, tag="gi")
            nc.vector.tensor_copy(gi[:], t[:])
            gf = ipool.tile([128, PTS_PER_PART * 3], F32, tag="gf")
            nc.vector.tensor_copy(gf[:], gi[:])
            corr = ipool.tile([128, PTS_PER_PART * 3], F32, tag="corr")
            nc.vector.tensor_tensor(
                out=corr[:], in0=gf[:], in1=t[:], op=mybir.AluOpType.is_gt)
            nc.vector.tensor_tensor(
                out=gf[:], in0=gf[:], in1=corr[:],
                op=mybir.AluOpType.subtract)

            # row = gx*S*S + gy*S + gz (exact in f32)
            g3 = gf[:].rearrange("p (w t) -> p w t", t=3)
            t1 = ipool.tile([128, PTS_PER_PART], F32, tag="t1")
            nc.vector.scalar_tensor_tensor(
                out=t1[:], in0=g3[:, :, 1], scalar=float(S),
                in1=g3[:, :, 2],
                op0=mybir.AluOpType.mult, op1=mybir.AluOpType.add)
            base = ipool.tile([128, PTS_PER_PART], F32, tag="base")
            nc.vector.scalar_tensor_tensor(
                out=base[:], in0=g3[:, :, 0], scalar=float(S * S),
                in1=t1[:],
                op0=mybir.AluOpType.mult, op1=mybir.AluOpType.add)

            lin = ipool.tile([128, PTS_PER_PART], I32, tag="lin")
            nc.vector.tensor_copy(lin[:], base[:])

            for g in range(GROUPS):
                gt = gpool.tile([128, UPG * ROW], I8, tag="gt")
                for jj in range(UPG):
                    j = g * UPG + jj
                    nc.gpsimd.indirect_dma_start(
                        out=gt[:, jj * ROW:(jj + 1) * ROW],
                        out_offset=None,
                        in_=table[:],
                        in_offset=bass.IndirectOffsetOnAxis(
                            ap=lin[:, j:j + 1], axis=0),
                    )
                nc.sync.dma_start(
                    out=outv[:, g * UPG * ROW:(g + 1) * UPG * ROW],
                    in_=gt[:])

    _legalize_single_wait(nc)
    return nc


def _build_table(cubic_b):
    """[C,S,S,S] -> corner-blocked table [S^3, 8*C] int8 + per-row scale."""
    pad = np.zeros((S + 1, S + 1, S + 1, C), dtype=np.float32)
    pad[:S, :S, :S] = np.transpose(cubic_b, (1, 2, 3, 0))
    t = np.empty((S, S, S, K, C), dtype=np.float32)
    for k in range(K):
        dx, dy, dz = (k >> 2) & 1, (k >> 1) & 1, k & 1
        t[:, :, :, k] = pad[dx:S + dx, dy:S + dy, dz:S + dz]
    t = t.reshape(TR, ROW)
    scale = np.maximum(np.abs(t).max(axis=1), 1e-20) / 127.0
    q = np.rint(t / scale[:, None]).astype(np.int8)
    return q, scale.astype(np.float32)


def _point_rows(ptcloud):
    """Per-point table row index, f32 math identical to the reference."""
    t = ptcloud.astype(np.float32) * np.float32(S / 2.0) + np.float32(S / 2.0)
    lower = np.floor(t).astype(np.int32)          # [B,N,3]
    return (lower[..., 0] * S + lower[..., 1]) * S + lower[..., 2]  # [B,N]


def _plan_blocks(lin_core):
    """Pack this shard's points into G blocks of P same-row points.
    Returns (asg [128,T] i32, dev_row [NP] i64, order [NP] i64, G) with
    srt[dev_row[j]] holding sorted-point j's row, or None if G > G_CAP."""
    order = np.argsort(lin_core, kind="stable")
    ls = lin_core[order]
    vals, counts = np.unique(ls, return_counts=True)
    nb = -(-counts // P)                          # ceil(counts/P) per row
    G = int(nb.sum())
    if G > G_CAP:
        return None
    bstart = np.concatenate(([0], np.cumsum(nb[:-1])))
    ostart = np.concatenate(([0], np.cumsum(counts[:-1])))
    q = np.arange(NP, dtype=np.int64) - np.repeat(ostart, counts)
    dev_row = (np.repeat(bstart, counts) + q // P) * P + q % P
    asg_flat = np.zeros(G_CAP, dtype=np.int32)
    asg_flat[:G] = np.repeat(vals, nb)
    asg = np.ascontiguousarray(asg_flat.reshape(T, 128).T)  # [128, T]
    return asg, dev_row, order, G


def _shard_inputs(ptcloud, cubic_features):
    """Build per-core input maps. Returns (in_maps, scales, plans, lin)."""
    ptcloud = np.ascontiguousarray(ptcloud, dtype=np.float32)
    cubic_features = np.asarray(cubic_features, dtype=np.float32)
    lin = _point_rows(ptcloud)
    half = N // 2
    in_maps, scales, plans = [], [], []
    for b in range(B):
        tb, sc = _build_table(cubic_features[b])
        scales.append(sc)
        for h in range(2):
            plan = _plan_blocks(lin[b, h * half:(h + 1) * half])
            plans.append(plan)
            if plan is None:
                in_maps.append({
                    "pts": np.ascontiguousarray(
                        ptcloud[b, h * half:(h + 1) * half]),
                    "table": tb,
                })
            else:
                in_maps.append({"table": tb, "asg": plan[0]})
    return in_maps, scales, plans, lin


def _gather_output(results, scales, plans, lin):
    half = N // 2
    out = np.empty((B, N, K, C), dtype=np.float32)
    for ci, r in enumerate(results):
        b, h = divmod(ci, 2)
        sl = slice(h * half, (h + 1) * half)
        plan = plans[ci]
        if plan is None:
            rowscale = scales[b][lin[b, sl]]
            deq = r["out"].reshape(half, K * C).astype(np.float32)
            deq *= rowscale[:, None]
            out[b, sl] = deq.reshape(half, K, C)
        else:
            _, dev_row, order, _ = plan
            deq = r["srt"][dev_row].astype(np.float32)     # [half, 2048]
            deq *= scales[b][lin[b, sl][order]][:, None]
            shard = np.empty((half, K * C), dtype=np.float32)
            shard[order] = deq
            out[b, sl] = shard.reshape(half, K, C)
    return out


def run(ptcloud, cubic_features, trace=False):
    """Shard, run on 8 cores, unshard. Returns (output, BassKernelResults)."""
    in_maps, scales, plans, lin = _shard_inputs(ptcloud, cubic_features)
    if all(p is not None for p in plans):
        nc = build_bass_blocks()
    else:
        # mixed shards would need two programs; run everything per-point
        nc = build_bass_points()
        rebuilt = []
        for ci, m in enumerate(in_maps):
            if "pts" not in m:
                b, h = divmod(ci, 2)
                half = N // 2
                m = {"pts": np.ascontiguousarray(
                        np.asarray(ptcloud, np.float32)[
                            b, h * half:(h + 1) * half]),
                     "table": m["table"]}
            rebuilt.append(m)
            plans[ci] = None
        in_maps = rebuilt
    res = run_bass_kernel_spmd(
        nc, in_maps, core_ids=list(range(N_CORES)), trace=trace)
    return _gather_output(res.results, scales, plans, lin), res


def kernel(ptcloud, cubic_features, neighborhood_size):
    assert int(neighborhood_size) == 1
    out, _ = run(ptcloud, cubic_features)
    return out


# revision 16
# speedup vs baseline: 1.0655x; 1.0655x over previous
"""CubicFeatureSampling Trainium2 kernel.

Full-input contract: kernel(ptcloud, cubic_features, neighborhood_size) with
  ptcloud:        [B=4, N=8192, 3]   f32 in [-1, 1]
  cubic_features: [B=4, C=256, S=32, S, S] f32
  neighborhood_size: 1
returns [B, N, K=8, C] f32 (rel err ~8e-3 vs the f32 jax reference,
well inside the 2e-2 gate).

Strategy (8 NeuronCores): data-parallel over (batch, half-of-N); each core
handles 4096 points against its batch's feature volume. The kernel is pure
HBM byte movement, so both optimizations shrink bytes:

1. int8 corner-blocked table. The host re-lays the volume as a zero-padded,
   channel-last table  table[(x*S+y)*S+z] = [f(x+dx,y+dy,z+dz) for
   k=dx*4+dy*2+dz], quantized to int8 with one f32 scale per row
   (scale = max|row|/127), shape [32768, 2048] (2KB rows; 0 where a coord
   hits S). Each point's whole [8, C] output block is ONE 2KB row, already
   in the reference's corner order; out-of-bounds corners are exact zeros.
   The host dequantizes (out_i8 * scale[row]) while unsharding.

2. Sorted-block replication. The point clouds are heavily clustered
   (~144 unique cells per 4096-point shard, ~28 points/cell), so the host
   sorts points by cell and packs them into G blocks of P=6 points that
   share one table row (G <= 768 = T*128 with T=6; padded blocks repeat
   the row and are simply ignored when unsharding). The device then runs,
   per tile t of 128 blocks:
     - ONE indirect SWDGE gather (the HW-verified "offset [128,1] + flat
       dest [128, X]" form): partition p <- table row asg[p, t] (2KB),
     - ONE HWDGE store replicating each partition's 2KB row P times into
       the block-sorted output srt[(t*128+p)*P + r] via a 0-stride
       broadcast source AP.
   That is 6 gathers (768 descriptors) + 6 stores per core: ~1.6 MB read +
   ~9.4 MB written vs 8.4+8.4 MB for a per-point gather, and ~10x fewer
   SWDGE descriptors (whose ~10ns/descriptor emission was the previous
   bottleneck). The host scatters srt rows back to per-point order while
   dequantizing (pure unshard-side bookkeeping).

If a shard is NOT clustered enough (G > 768), fall back to the per-point
indirect-gather kernel (32 gathers of [128,1] + 4 stores, 78 us).
"""

import numpy as np

import concourse.bass as bass
import concourse.tile as tile
from concourse import mybir
from concourse.bass_utils import run_bass_kernel_spmd

# Problem constants (hardcoded per harness contract).
B = 4
N = 8192
C = 256
S = 32
K = 8
N_CORES = 8
NP = (B * N) // N_CORES   # points per core = 4096

TR = S * S * S            # table rows (32768)
ROW = K * C               # 2048 elements (2KB in int8) per table row

# Block-replication path constants.
P = 5                     # points per block (replication factor)
T = 7                     # tiles of 128 blocks
G_CAP = T * 128           # max blocks per core (896)

# Per-point fallback path constants.
PTS_PER_PART = NP // 128  # 32 points per partition
UPG = 8                   # points (gather units) per store group
GROUPS = PTS_PER_PART // UPG  # 4 groups

F32 = mybir.dt.float32
I8 = mybir.dt.int8
I32 = mybir.dt.int32


def _legalize_single_wait(nc):
    """The walrus build here accepts exactly ONE sync wait per instruction
    (setupSyncWait: 'Too many sync wait commands'), but Tile's add_semaphores
    pass emits up to ~3 on DMAs and the tail drain. Hoist all but the last
    wait of each instruction into standalone same-engine InstEventSemaphore
    waits placed immediately before it — semantically identical (the engine
    queue is processed in order)."""
    f = nc.m.functions[0]
    for b in f.blocks:
        out, changed = [], False
        for inst in b.instructions:
            si = inst.sync_info
            if si is not None and si.on_wait and len(si.on_wait) > 1:
                waits = list(si.on_wait)
                for w in waits[:-1]:
                    ev = mybir.InstEventSemaphore(
                        name=nc.get_next_instruction_name(), ins=[], outs=[])
                    ev.engine = inst.engine
                    ev.sync_info = mybir.SyncInfo(on_wait=[w], on_update=[])
                    nc.register_instruction(ev, overwrite=True)
                    out.append(ev)
                inst.sync_info = mybir.SyncInfo(
                    on_wait=[waits[-1]], on_update=list(si.on_update or []))
                changed = True
            out.append(inst)
        if changed:
            b.instructions = out


def build_bass_blocks():
    """Sorted-block replication kernel: T gathers + T replicating stores."""
    nc = bass.Bass("TRN2")
    table = nc.declare_dram_parameter("table", [TR, ROW], I8, isOutput=False)
    asg = nc.declare_dram_parameter("asg", [128, T], I32, isOutput=False)
    srt = nc.declare_dram_parameter("srt", [G_CAP * P, ROW], I8,
                                    isOutput=True)

    # Block g = t*128 + p occupies srt rows [g*P, (g+1)*P): tile t's
    # 128*P rows are contiguous, P*ROW bytes per partition.

    H = P // 2                # sync-queue store rows (2; scalar gets 3)

    with tile.TileContext(nc) as tc:
        with (
            tc.tile_pool(name="gather", bufs=4) as gpool,
            tc.tile_pool(name="rep", bufs=4) as rpool,
            tc.tile_pool(name="idx", bufs=1) as ipool,
        ):
            asg_sb = ipool.tile([128, T], I32, tag="asg")
            nc.sync.dma_start(out=asg_sb[:], in_=asg[:])

            for t in range(T):
                gt = gpool.tile([128, ROW], I8, tag="gt")
                nc.gpsimd.indirect_dma_start(
                    out=gt[:],
                    out_offset=None,
                    in_=table[:],
                    in_offset=bass.IndirectOffsetOnAxis(
                        ap=asg_sb[:, t:t + 1], axis=0),
                )
                # Replicate each partition's 2KB row P times in SBUF (DVE,
                # i32-bitcast for 4x fewer elements) so the stores move
                # contiguous multi-row descriptors at full engine rate.
                gtP = rpool.tile([128, P * ROW], I8, tag="gtP")
                nc.vector.tensor_copy(
                    gtP[:].bitcast(I32).rearrange(
                        "p (r d) -> p r d", r=P),
                    gt[:].bitcast(I32).unsqueeze(1).broadcast_to(
                        (128, P, ROW // 4)))
                # Tile t's srt rows, viewed [p][r][d]; store the two
                # row-halves on the two HWDGE queues for finer overlap.
                srt_t = srt[t * 128 * P:(t + 1) * 128 * P, :].rearrange(
                    "(p r) d -> p r d", p=128)
                nc.sync.dma_start(
                    out=srt_t[:, 0:H, :], in_=gtP[:, 0:H * ROW])
                nc.scalar.dma_start(
                    out=srt_t[:, H:P, :], in_=gtP[:, H * ROW:P * ROW])

    _legalize_single_wait(nc)
    return nc


def build_bass_points():
    """Per-point fallback: 32 indirect gathers + 4 stores, on-device index
    computation from pts."""
    nc = bass.Bass("TRN2")
    pts = nc.declare_dram_parameter("pts", [NP, 3], F32, isOutput=False)
    table = nc.declare_dram_parameter("table", [TR, ROW], I8, isOutput=False)
    out = nc.declare_dram_parameter("out", [NP * K, C], I8, isOutput=True)

    outv = out[:].rearrange("(p u) d -> p (u d)", p=128)  # [128, 256*C]

    with tile.TileContext(nc) as tc:
        with (
            tc.tile_pool(name="gather", bufs=2) as gpool,
            tc.tile_pool(name="idx", bufs=1) as ipool,
        ):
            pt_all = ipool.tile([128, PTS_PER_PART * 3], F32, tag="ptall")
            nc.sync.dma_start(
                out=pt_all[:],
                in_=pts[:].rearrange("(p w) t -> p (w t)", p=128))

            # t = pt*16 + 16 (one f32 rounding, identical to the reference)
            t = ipool.tile([128, PTS_PER_PART * 3], F32, tag="t")
            nc.scalar.activation(
                out=t[:], in_=pt_all[:],
                func=mybir.ActivationFunctionType.Copy,
                bias=float(S) / 2.0, scale=float(S) / 2.0)

            # exact floor(t), robust to the f32->i32 rounding mode
            gi = ipool.tile([128, PTS_PER_PART * 3], I32, tag="gi")
            nc.vector.tensor_copy(gi[:], t[:])
            gf = ipool.tile([128, PTS_PER_PART * 3], F32, tag="gf")
            nc.vector.tensor_copy(gf[:], gi[:])
            corr = ipool.tile([128, PTS_PER_PART * 3], F32, tag="corr")
            nc.vector.tensor_tensor(
                out=corr[:], in0=gf[:], in1=t[:], op=mybir.AluOpType.is_gt)
            nc.vector.tensor_tensor(
                out=gf[:], in0=gf[:], in1=corr[:],
                op=mybir.AluOpType.subtract)

            # row = gx*S*S + gy*S + gz (exact in f32)
            g3 = gf[:].rearrange("p (w t) -> p w t", t=3)
            t1 = ipool.tile([128, PTS_PER_PART], F32, tag="t1")
            nc.vector.scalar_tensor_tensor(
                out=t1[:], in0=g3[:, :, 1], scalar=float(S),
                in1=g3[:, :, 2],
                op0=mybir.AluOpType.mult, op1=mybir.AluOpType.add)
            base = ipool.tile([128, PTS_PER_PART], F32, tag="base")
            nc.vector.scalar_tensor_tensor(
                out=base[:], in0=g3[:, :, 0], scalar=float(S * S),
                in1=t1[:],
                op0=mybir.AluOpType.mult, op1=mybir.AluOpType.add)

            lin = ipool.tile([128, PTS_PER_PART], I32, tag="lin")
            nc.vector.tensor_copy(lin[:], base[:])

            for g in range(GROUPS):
                gt = gpool.tile([128, UPG * ROW], I8, tag="gt")
                for jj in range(UPG):
                    j = g * UPG + jj
                    nc.gpsimd.indirect_dma_start(
                        out=gt[:, jj * ROW:(jj + 1) * ROW],
                        out_offset=None,
                        in_=table[:],
                        in_offset=bass.IndirectOffsetOnAxis(
                            ap=lin[:, j:j + 1], axis=0),
                    )
                nc.sync.dma_start(
                    out=outv[:, g * UPG * ROW:(g + 1) * UPG * ROW],
                    in_=gt[:])

    _legalize_single_wait(nc)
    return nc


def _build_table(cubic_b):
    """[C,S,S,S] -> corner-blocked table [S^3, 8*C] int8 + per-row scale."""
    pad = np.zeros((S + 1, S + 1, S + 1, C), dtype=np.float32)
    pad[:S, :S, :S] = np.transpose(cubic_b, (1, 2, 3, 0))
    t = np.empty((S, S, S, K, C), dtype=np.float32)
    for k in range(K):
        dx, dy, dz = (k >> 2) & 1, (k >> 1) & 1, k & 1
        t[:, :, :, k] = pad[dx:S + dx, dy:S + dy, dz:S + dz]
    t = t.reshape(TR, ROW)
    scale = np.maximum(np.abs(t).max(axis=1), 1e-20) / 127.0
    q = np.rint(t / scale[:, None]).astype(np.int8)
    return q, scale.astype(np.float32)


def _point_rows(ptcloud):
    """Per-point table row index, f32 math identical to the reference."""
    t = ptcloud.astype(np.float32) * np.float32(S / 2.0) + np.float32(S / 2.0)
    lower = np.floor(t).astype(np.int32)          # [B,N,3]
    return (lower[..., 0] * S + lower[..., 1]) * S + lower[..., 2]  # [B,N]


def _plan_blocks(lin_core):
    """Pack this shard's points into G blocks of P same-row points.
    Returns (asg [128,T] i32, dev_row [NP] i64, order [NP] i64, G) with
    srt[dev_row[j]] holding sorted-point j's row, or None if G > G_CAP."""
    order = np.argsort(lin_core, kind="stable")
    ls = lin_core[order]
    vals, counts = np.unique(ls, return_counts=True)
    nb = -(-counts // P)                          # ceil(counts/P) per row
    G = int(nb.sum())
    if G > G_CAP:
        return None
    bstart = np.concatenate(([0], np.cumsum(nb[:-1])))
    ostart = np.concatenate(([0], np.cumsum(counts[:-1])))
    q = np.arange(NP, dtype=np.int64) - np.repeat(ostart, counts)
    dev_row = (np.repeat(bstart, counts) + q // P) * P + q % P
    asg_flat = np.zeros(G_CAP, dtype=np.int32)
    asg_flat[:G] = np.repeat(vals, nb)
    asg = np.ascontiguousarray(asg_flat.reshape(T, 128).T)  # [128, T]
    return asg, dev_row, order, G


def _shard_inputs(ptcloud, cubic_features):
    """Build per-core input maps. Returns (in_maps, scales, plans, lin)."""
    ptcloud = np.ascontiguousarray(ptcloud, dtype=np.float32)
    cubic_features = np.asarray(cubic_features, dtype=np.float32)
    lin = _point_rows(ptcloud)
    half = N // 2
    in_maps, scales, plans = [], [], []
    for b in range(B):
        tb, sc = _build_table(cubic_features[b])
        scales.append(sc)
        for h in range(2):
            plan = _plan_blocks(lin[b, h * half:(h + 1) * half])
            plans.append(plan)
            if plan is None:
                in_maps.append({
                    "pts": np.ascontiguousarray(
                        ptcloud[b, h * half:(h + 1) * half]),
                    "table": tb,
                })
            else:
                in_maps.append({"table": tb, "asg": plan[0]})
    return in_maps, scales, plans, lin


def _gather_output(results, scales, plans, lin):
    half = N // 2
    out = np.empty((B, N, K, C), dtype=np.float32)
    for ci, r in enumerate(results):
        b, h = divmod(ci, 2)
        sl = slice(h * half, (h + 1) * half)
        plan = plans[ci]
        if plan is None:
            rowscale = scales[b][lin[b, sl]]
            deq = r["out"].reshape(half, K * C).astype(np.float32)
            deq *= rowscale[:, None]
            out[b, sl] = deq.reshape(half, K, C)
        else:
            _, dev_row, order, _ = plan
            deq = r["srt"][dev_row].astype(np.float32)     # [half, 2048]
            deq *= scales[b][lin[b, sl][order]][:, None]
            shard = np.empty((half, K * C), dtype=np.float32)
            shard[order] = deq
            out[b, sl] = shard.reshape(half, K, C)
    return out


def run(ptcloud, cubic_features, trace=False):
    """Shard, run on 8 cores, unshard. Returns (output, BassKernelResults)."""
    in_maps, scales, plans, lin = _shard_inputs(ptcloud, cubic_features)
    if all(p is not None for p in plans):
        nc = build_bass_blocks()
    else:
        # mixed shards would need two programs; run everything per-point
        nc = build_bass_points()
        rebuilt = []
        for ci, m in enumerate(in_maps):
            if "pts" not in m:
                b, h = divmod(ci, 2)
                half = N // 2
                m = {"pts": np.ascontiguousarray(
                        np.asarray(ptcloud, np.float32)[
                            b, h * half:(h + 1) * half]),
                     "table": m["table"]}
            rebuilt.append(m)
            plans[ci] = None
        in_maps = rebuilt
    res = run_bass_kernel_spmd(
        nc, in_maps, core_ids=list(range(N_CORES)), trace=trace)
    return _gather_output(res.results, scales, plans, lin), res


def kernel(ptcloud, cubic_features, neighborhood_size):
    assert int(neighborhood_size) == 1
    out, _ = run(ptcloud, cubic_features)
    return out


# revision 17
# speedup vs baseline: 1.1808x; 1.1082x over previous
"""CubicFeatureSampling Trainium2 kernel.

Full-input contract: kernel(ptcloud, cubic_features, neighborhood_size) with
  ptcloud:        [B=4, N=8192, 3]   f32 in [-1, 1]
  cubic_features: [B=4, C=256, S=32, S, S] f32
  neighborhood_size: 1
returns [B, N, K=8, C] f32 (rel err ~8e-3 vs the f32 jax reference,
well inside the 2e-2 gate).

Strategy (8 NeuronCores): data-parallel over (batch, half-of-N); each core
handles 4096 points against its batch's feature volume. The kernel is pure
HBM byte movement, so both optimizations shrink bytes:

1. int8 corner-blocked table. The host re-lays the volume as a zero-padded,
   channel-last table  table[(x*S+y)*S+z] = [f(x+dx,y+dy,z+dz) for
   k=dx*4+dy*2+dz], quantized to int8 with one f32 scale per row
   (scale = max|row|/127), shape [32768, 2048] (2KB rows; 0 where a coord
   hits S). Each point's whole [8, C] output block is ONE 2KB row, already
   in the reference's corner order; out-of-bounds corners are exact zeros.
   The host dequantizes (out_i8 * scale[row]) while unsharding.

2. Sorted-block replication. The point clouds are heavily clustered
   (~144 unique cells per 4096-point shard, ~28 points/cell), so the host
   sorts points by cell and packs them into G blocks of P=6 points that
   share one table row (G <= 768 = T*128 with T=6; padded blocks repeat
   the row and are simply ignored when unsharding). The device then runs,
   per tile t of 128 blocks:
     - ONE indirect SWDGE gather (the HW-verified "offset [128,1] + flat
       dest [128, X]" form): partition p <- table row asg[p, t] (2KB),
     - ONE HWDGE store replicating each partition's 2KB row P times into
       the block-sorted output srt[(t*128+p)*P + r] via a 0-stride
       broadcast source AP.
   That is 6 gathers (768 descriptors) + 6 stores per core: ~1.6 MB read +
   ~9.4 MB written vs 8.4+8.4 MB for a per-point gather, and ~10x fewer
   SWDGE descriptors (whose ~10ns/descriptor emission was the previous
   bottleneck). The host scatters srt rows back to per-point order while
   dequantizing (pure unshard-side bookkeeping).

If a shard is NOT clustered enough (G > 768), fall back to the per-point
indirect-gather kernel (32 gathers of [128,1] + 4 stores, 78 us).
"""

import numpy as np

import concourse.bass as bass
import concourse.tile as tile
from concourse import mybir
from concourse.bass_utils import run_bass_kernel_spmd

# Problem constants (hardcoded per harness contract).
B = 4
N = 8192
C = 256
S = 32
K = 8
N_CORES = 8
NP = (B * N) // N_CORES   # points per core = 4096

TR = S * S * S            # table rows (32768)
ROW = K * C               # 2048 elements (2KB in int8) per table row

# Block-replication path constants.
P = 6                     # points per block (replication factor)
T = 6                     # tiles of 128 blocks
G_CAP = T * 128           # max blocks per core (768)

# Per-point fallback path constants.
PTS_PER_PART = NP // 128  # 32 points per partition
UPG = 8                   # points (gather units) per store group
GROUPS = PTS_PER_PART // UPG  # 4 groups

F32 = mybir.dt.float32
I8 = mybir.dt.int8
I32 = mybir.dt.int32


def _legalize_single_wait(nc):
    """The walrus build here accepts exactly ONE sync wait per instruction
    (setupSyncWait: 'Too many sync wait commands'), but Tile's add_semaphores
    pass emits up to ~3 on DMAs and the tail drain. Hoist all but the last
    wait of each instruction into standalone same-engine InstEventSemaphore
    waits placed immediately before it — semantically identical (the engine
    queue is processed in order)."""
    f = nc.m.functions[0]
    for b in f.blocks:
        out, changed = [], False
        for inst in b.instructions:
            si = inst.sync_info
            if si is not None and si.on_wait and len(si.on_wait) > 1:
                waits = list(si.on_wait)
                for w in waits[:-1]:
                    ev = mybir.InstEventSemaphore(
                        name=nc.get_next_instruction_name(), ins=[], outs=[])
                    ev.engine = inst.engine
                    ev.sync_info = mybir.SyncInfo(on_wait=[w], on_update=[])
                    nc.register_instruction(ev, overwrite=True)
                    out.append(ev)
                inst.sync_info = mybir.SyncInfo(
                    on_wait=[waits[-1]], on_update=list(si.on_update or []))
                changed = True
            out.append(inst)
        if changed:
            b.instructions = out


def build_bass_blocks():
    """Sorted-block replication kernel: T gathers + T replicating stores."""
    nc = bass.Bass("TRN2")
    table = nc.declare_dram_parameter("table", [TR, ROW], I8, isOutput=False)
    asg = nc.declare_dram_parameter("asg", [128, T], I32, isOutput=False)
    srt = nc.declare_dram_parameter("srt", [G_CAP * P, ROW], I8,
                                    isOutput=True)

    # Block g = t*128 + p occupies srt rows [g*P, (g+1)*P): tile t's
    # 128*P rows are contiguous, P*ROW bytes per partition.

    H = P // 2                # sync-queue store rows

    with tile.TileContext(nc) as tc:
        with (
            tc.tile_pool(name="gather", bufs=3) as gpool,
            tc.tile_pool(name="rep", bufs=3) as rpool,
            tc.tile_pool(name="idx", bufs=1) as ipool,
        ):
            asg_sb = ipool.tile([128, T], I32, tag="asg")
            nc.sync.dma_start(out=asg_sb[:], in_=asg[:])

            for t in range(T):
                gt = gpool.tile([128, ROW], I8, tag="gt")
                nc.gpsimd.indirect_dma_start(
                    out=gt[:],
                    out_offset=None,
                    in_=table[:],
                    in_offset=bass.IndirectOffsetOnAxis(
                        ap=asg_sb[:, t:t + 1], axis=0),
                )
                # Replicate each partition's 2KB row P times in SBUF (DVE,
                # i32-bitcast for 4x fewer elements) so the stores move
                # contiguous multi-row descriptors at full engine rate.
                gtP = rpool.tile([128, P * ROW], I8, tag="gtP")
                nc.vector.tensor_copy(
                    gtP[:].bitcast(I32).rearrange(
                        "p (r d) -> p r d", r=P),
                    gt[:].bitcast(I32).unsqueeze(1).broadcast_to(
                        (128, P, ROW // 4)))
                # Tile t's srt rows, viewed [p][r][d]; store the two
                # row-halves on the two HWDGE queues for finer overlap.
                srt_t = srt[t * 128 * P:(t + 1) * 128 * P, :].rearrange(
                    "(p r) d -> p r d", p=128)
                nc.sync.dma_start(
                    out=srt_t[:, 0:H, :], in_=gtP[:, 0:H * ROW])
                nc.scalar.dma_start(
                    out=srt_t[:, H:P, :], in_=gtP[:, H * ROW:P * ROW])

    _legalize_single_wait(nc)
    return nc


def build_bass_points():
    """Per-point fallback: 32 indirect gathers + 4 stores, on-device index
    computation from pts."""
    nc = bass.Bass("TRN2")
    pts = nc.declare_dram_parameter("pts", [NP, 3], F32, isOutput=False)
    table = nc.declare_dram_parameter("table", [TR, ROW], I8, isOutput=False)
    out = nc.declare_dram_parameter("out", [NP * K, C], I8, isOutput=True)

    outv = out[:].rearrange("(p u) d -> p (u d)", p=128)  # [128, 256*C]

    with tile.TileContext(nc) as tc:
        with (
            tc.tile_pool(name="gather", bufs=2) as gpool,
            tc.tile_pool(name="idx", bufs=1) as ipool,
        ):
            pt_all = ipool.tile([128, PTS_PER_PART * 3], F32, tag="ptall")
            nc.sync.dma_start(
                out=pt_all[:],
                in_=pts[:].rearrange("(p w) t -> p (w t)", p=128))

            # t = pt*16 + 16 (one f32 rounding, identical to the reference)
            t = ipool.tile([128, PTS_PER_PART * 3], F32, tag="t")
            nc.scalar.activation(
                out=t[:], in_=pt_all[:],
                func=mybir.ActivationFunctionType.Copy,
                bias=float(S) / 2.0, scale=float(S) / 2.0)

            # exact floor(t), robust to the f32->i32 rounding mode
            gi = ipool.tile([128, PTS_PER_PART * 3], I32, tag="gi")
            nc.vector.tensor_copy(gi[:], t[:])
            gf = ipool.tile([128, PTS_PER_PART * 3], F32, tag="gf")
            nc.vector.tensor_copy(gf[:], gi[:])
            corr = ipool.tile([128, PTS_PER_PART * 3], F32, tag="corr")
            nc.vector.tensor_tensor(
                out=corr[:], in0=gf[:], in1=t[:], op=mybir.AluOpType.is_gt)
            nc.vector.tensor_tensor(
                out=gf[:], in0=gf[:], in1=corr[:],
                op=mybir.AluOpType.subtract)

            # row = gx*S*S + gy*S + gz (exact in f32)
            g3 = gf[:].rearrange("p (w t) -> p w t", t=3)
            t1 = ipool.tile([128, PTS_PER_PART], F32, tag="t1")
            nc.vector.scalar_tensor_tensor(
                out=t1[:], in0=g3[:, :, 1], scalar=float(S),
                in1=g3[:, :, 2],
                op0=mybir.AluOpType.mult, op1=mybir.AluOpType.add)
            base = ipool.tile([128, PTS_PER_PART], F32, tag="base")
            nc.vector.scalar_tensor_tensor(
                out=base[:], in0=g3[:, :, 0], scalar=float(S * S),
                in1=t1[:],
                op0=mybir.AluOpType.mult, op1=mybir.AluOpType.add)

            lin = ipool.tile([128, PTS_PER_PART], I32, tag="lin")
            nc.vector.tensor_copy(lin[:], base[:])

            for g in range(GROUPS):
                gt = gpool.tile([128, UPG * ROW], I8, tag="gt")
                for jj in range(UPG):
                    j = g * UPG + jj
                    nc.gpsimd.indirect_dma_start(
                        out=gt[:, jj * ROW:(jj + 1) * ROW],
                        out_offset=None,
                        in_=table[:],
                        in_offset=bass.IndirectOffsetOnAxis(
                            ap=lin[:, j:j + 1], axis=0),
                    )
                nc.sync.dma_start(
                    out=outv[:, g * UPG * ROW:(g + 1) * UPG * ROW],
                    in_=gt[:])

    _legalize_single_wait(nc)
    return nc


def _build_table(cubic_b):
    """[C,S,S,S] -> corner-blocked table [S^3, 8*C] int8 + per-row scale."""
    pad = np.zeros((S + 1, S + 1, S + 1, C), dtype=np.float32)
    pad[:S, :S, :S] = np.transpose(cubic_b, (1, 2, 3, 0))
    t = np.empty((S, S, S, K, C), dtype=np.float32)
    for k in range(K):
        dx, dy, dz = (k >> 2) & 1, (k >> 1) & 1, k & 1
        t[:, :, :, k] = pad[dx:S + dx, dy:S + dy, dz:S + dz]
    t = t.reshape(TR, ROW)
    scale = np.maximum(np.abs(t).max(axis=1), 1e-20) / 127.0
    q = np.rint(t / scale[:, None]).astype(np.int8)
    return q, scale.astype(np.float32)


def _point_rows(ptcloud):
    """Per-point table row index, f32 math identical to the reference."""
    t = ptcloud.astype(np.float32) * np.float32(S / 2.0) + np.float32(S / 2.0)
    lower = np.floor(t).astype(np.int32)          # [B,N,3]
    return (lower[..., 0] * S + lower[..., 1]) * S + lower[..., 2]  # [B,N]


def _plan_blocks(lin_core):
    """Pack this shard's points into G blocks of P same-row points.
    Returns (asg [128,T] i32, dev_row [NP] i64, order [NP] i64, G) with
    srt[dev_row[j]] holding sorted-point j's row, or None if G > G_CAP."""
    order = np.argsort(lin_core, kind="stable")
    ls = lin_core[order]
    vals, counts = np.unique(ls, return_counts=True)
    nb = -(-counts // P)                          # ceil(counts/P) per row
    G = int(nb.sum())
    if G > G_CAP:
        return None
    bstart = np.concatenate(([0], np.cumsum(nb[:-1])))
    ostart = np.concatenate(([0], np.cumsum(counts[:-1])))
    q = np.arange(NP, dtype=np.int64) - np.repeat(ostart, counts)
    dev_row = (np.repeat(bstart, counts) + q // P) * P + q % P
    asg_flat = np.zeros(G_CAP, dtype=np.int32)
    asg_flat[:G] = np.repeat(vals, nb)
    asg = np.ascontiguousarray(asg_flat.reshape(T, 128).T)  # [128, T]
    return asg, dev_row, order, G


def _shard_inputs(ptcloud, cubic_features):
    """Build per-core input maps. Returns (in_maps, scales, plans, lin)."""
    ptcloud = np.ascontiguousarray(ptcloud, dtype=np.float32)
    cubic_features = np.asarray(cubic_features, dtype=np.float32)
    lin = _point_rows(ptcloud)
    half = N // 2
    in_maps, scales, plans = [], [], []
    for b in range(B):
        tb, sc = _build_table(cubic_features[b])
        scales.append(sc)
        for h in range(2):
            plan = _plan_blocks(lin[b, h * half:(h + 1) * half])
            plans.append(plan)
            if plan is None:
                in_maps.append({
                    "pts": np.ascontiguousarray(
                        ptcloud[b, h * half:(h + 1) * half]),
                    "table": tb,
                })
            else:
                in_maps.append({"table": tb, "asg": plan[0]})
    return in_maps, scales, plans, lin


def _gather_output(results, scales, plans, lin):
    half = N // 2
    out = np.empty((B, N, K, C), dtype=np.float32)
    for ci, r in enumerate(results):
        b, h = divmod(ci, 2)
        sl = slice(h * half, (h + 1) * half)
        plan = plans[ci]
        if plan is None:
            rowscale = scales[b][lin[b, sl]]
            deq = r["out"].reshape(half, K * C).astype(np.float32)
            deq *= rowscale[:, None]
            out[b, sl] = deq.reshape(half, K, C)
        else:
            _, dev_row, order, _ = plan
            deq = r["srt"][dev_row].astype(np.float32)     # [half, 2048]
            deq *= scales[b][lin[b, sl][order]][:, None]
            shard = np.empty((half, K * C), dtype=np.float32)
            shard[order] = deq
            out[b, sl] = shard.reshape(half, K, C)
    return out


def run(ptcloud, cubic_features, trace=False):
    """Shard, run on 8 cores, unshard. Returns (output, BassKernelResults)."""
    in_maps, scales, plans, lin = _shard_inputs(ptcloud, cubic_features)
    if all(p is not None for p in plans):
        nc = build_bass_blocks()
    else:
        # mixed shards would need two programs; run everything per-point
        nc = build_bass_points()
        rebuilt = []
        for ci, m in enumerate(in_maps):
            if "pts" not in m:
                b, h = divmod(ci, 2)
                half = N // 2
                m = {"pts": np.ascontiguousarray(
                        np.asarray(ptcloud, np.float32)[
                            b, h * half:(h + 1) * half]),
                     "table": m["table"]}
            rebuilt.append(m)
            plans[ci] = None
        in_maps = rebuilt
    res = run_bass_kernel_spmd(
        nc, in_maps, core_ids=list(range(N_CORES)), trace=trace)
    return _gather_output(res.results, scales, plans, lin), res


def kernel(ptcloud, cubic_features, neighborhood_size):
    assert int(neighborhood_size) == 1
    out, _ = run(ptcloud, cubic_features)
    return out
